# revision 41
# baseline (speedup 1.0000x reference)
"""CBL (contrastive boundary) loss kernel for Trainium2, 8 NeuronCores.

Strategy (data-parallel over points, per spec sharding hint):
  - Shard the N=100000 points across 8 cores (12500 each, zero-padded to
    12544 = 128 partitions x 98 tiles).
  - The original kernel issued one SWDGE indirect DMA per (tile, k) pair:
    686 instructions x (994 ns fixed + 128*0.34 ns) ~= 711 us of pure
    Pool-engine descriptor-generation overhead -- the measured bottleneck.
  - This version uses batched `dma_gather` (InstDMAGatherAnt). Its Q7 ucode
    stages indices in a 1024-entry data scratch, so one instruction moves at
    most 1024 rows; each core issues 86 sub-gathers (vs 686 indirect DMAs).
  - Sub-gathers round-robin over 4 SWDGE queues: each queue has its own
    descriptor ring (and Q7 cpu pair), so desc-gen of gather n does not
    block on the ring space freed only when gather n-1's DMA completes
    (measured: single-queue ping-pong cost ~6.9 us per gather).
  - dma_gather indexes are int16 (< 32768), so each 16-tile chunk gathers
    from its own host-compacted table: the distinct neighbor rows referenced
    by that chunk, remapped to local int16 ids ("neighbor_idx values stay
    local per-shard", as the sharding hint suggests). Compaction is pure
    index marshaling (np.unique/searchsorted); all float math stays on-chip.
  - Neighbor labels (target[neighbor_idx]) ship as a marshaled input the
    same way the point labels do; features are gathered on-chip.
  - Tables are bf16 (256B rows): halves gather bytes and enables the DVE
    2x_1p mode (all-2-byte packed operands) for the big elementwise ops.
    DVE tensor_reduce has no fast mode, so the C=128 reductions run as
    bf16 halving-add trees (2x rate), with the final 2->1 step emitting f32.
  - cos = dot * rsqrt(ss_i) * rsqrt(ss_j); dist = sqrt(max(2-2cos,0)+eps)
    -- identical to the reference's normalized L2 distance.
  - NCE contrast / masking / per-pair loss evaluated on-chip; each core
    emits partial (sum, count); host combines the 8 pairs (the scalar
    "all-reduce" of the sharding hint) and forms sum/max(cnt,1).
"""

import sys

if "/opt/trn_rl_repo" not in sys.path:
    sys.path.insert(0, "/opt/trn_rl_repo")

import numpy as np

N_TOTAL = 100000
C = 128
K = 7
NCORES = 8
P = 128
NSH = N_TOTAL // NCORES          # 12500 points per core
EPS = 1e-12
FP16 = False                     # kept for test.py compatibility

NQUEUES = 4                      # SWDGE queues used for gathers
GMAX = 1024                      # max idxs per dma_gather (Q7 scratch limit)
T_TOT = 98                       # tiles per core (12544 points)
NPAD = T_TOT * P                 # 12544
TK = T_TOT * K                   # 686
# chunk-size ramp: small chunks at the start (pipeline fill) and end
# (drain); 16-tile chunks in the middle (112 slots = 14 1024-idx gathers)
CHUNK_TILES = [8, 16, 16, 16, 16, 16, 8, 2]
NCHUNK = len(CHUNK_TILES)
CHUNK_T0 = np.cumsum([0] + CHUNK_TILES).tolist()


def build_nc(fp16=False):
    from concourse import bacc, bass  # noqa: F401
    import concourse.mybir as mybir
    from concourse.tile import TileContext

    f32 = mybir.dt.float32
    bf16 = mybir.dt.bfloat16
    i16 = mybir.dt.int16
    Alu = mybir.AluOpType
    Act = mybir.ActivationFunctionType
    Ax = mybir.AxisListType

    GGRP = max(CHUNK_TILES)      # 16 tiles per full chunk
    USZ = GGRP * K * P           # table rows per chunk (padded upper bound)

    nc = bacc.Bacc(num_devices=NCORES, num_swdge_queues=NQUEUES)
    xs = nc.dram_tensor("xs", [NPAD, C], bf16, kind="ExternalInput")
    tab = nc.dram_tensor("tab", [NCHUNK, USZ, C], bf16, kind="ExternalInput")
    idx16 = nc.dram_tensor("idx16", [NCHUNK, P, USZ // 16], i16,
                           kind="ExternalInput")
    tgn = nc.dram_tensor("tgn", [P, TK], f32, kind="ExternalInput")
    tgts = nc.dram_tensor("tgts", [P, T_TOT], f32, kind="ExternalInput")
    part = nc.dram_tensor("part", [2], f32, kind="ExternalOutput")

    qcnt = [0]

    def halving_tree(eng, buf, out2d):
        """Sum over the last (C) axis of buf [P, S, C] (bf16): in-place
        halving adds (DVE 2x_1p) down to 8 partials, then one native f32
        tensor_reduce into out2d [P, S]."""
        w = C // 2
        while w >= 16:
            eng.tensor_tensor(out=buf[:, :, 0:w], in0=buf[:, :, 0:w],
                              in1=buf[:, :, w:2 * w], op=Alu.add)
            w //= 2
        eng.tensor_reduce(out=out2d, in_=buf[:, :, 0:16], axis=Ax.X,
                          op=Alu.add)

    with TileContext(nc) as tc:
        with (
            tc.tile_pool(name="cst", bufs=1) as cst,
            tc.tile_pool(name="nbrp", bufs=3) as nbrp,
            tc.tile_pool(name="xsp", bufs=2) as xsp,
            tc.tile_pool(name="idxp", bufs=2) as idxp,
            tc.tile_pool(name="sqnp", bufs=2) as sqnp,
            tc.tile_pool(name="prodp", bufs=1) as prodp,
            tc.tile_pool(name="sqxp", bufs=2) as sqxp,
            tc.tile_pool(name="psp", bufs=1, space="PSUM") as psp,
        ):
            # ---- resident loads ----
            tgn_sb = cst.tile([P, TK], f32)
            tgts_sb = cst.tile([P, T_TOT], f32)
            nc.sync.dma_start(out=tgn_sb[:], in_=tgn[:, :])
            nc.sync.dma_start(out=tgts_sb[:], in_=tgts[:, :])

            dot_all = cst.tile([P, TK], f32)
            ssn_all = cst.tile([P, TK], f32)
            ssi_all = cst.tile([P, T_TOT], f32)

            # phase-B scratch (shared by both halves, disjoint slices)
            r_sb = cst.tile([P, T_TOT], f32)
            rn = cst.tile([P, TK], f32)
            e_t = cst.tile([P, TK], f32)
            M = cst.tile([P, T_TOT], f32)
            npos = cst.tile([P, T_TOT], f32)
            g1 = cst.tile([P, T_TOT], f32)
            pm = cst.tile([P, T_TOT], f32)
            sall = cst.tile([P, T_TOT], f32)
            spos = cst.tile([P, T_TOT], f32)
            eps_tile = cst.tile([P, 1], f32)
            nc.vector.memset(eps_tile[:], 2.0 + EPS)
            vals1 = cst.tile([P, 2], f32)
            vals2 = cst.tile([P, 2], f32)

            def seg_all(ap):
                return ap.rearrange("p (t k) -> p t k", k=K)

            # posmask / npos / point-mask depend only on labels: compute them
            # during pipeline fill. pos overwrites tgn_sb.
            pos = tgn_sb
            nc.vector.tensor_tensor(
                out=seg_all(pos[:]), in0=seg_all(tgn_sb[:]),
                in1=tgts_sb[:, :, None].to_broadcast([P, T_TOT, K]),
                op=Alu.is_equal)
            nc.vector.tensor_reduce(out=npos[:], in_=seg_all(pos[:]),
                                    axis=Ax.X, op=Alu.add)
            nc.vector.tensor_scalar(g1[:], npos[:], 0.5, None, Alu.is_gt)
            nc.vector.tensor_scalar(pm[:], npos[:], K - 0.5, None, Alu.is_lt)
            nc.vector.tensor_tensor(out=pm[:], in0=g1[:], in1=pm[:], op=Alu.mult)

            def emit_phase_b(ta, tb, vals):
                """Per-pair loss for tiles [ta, tb) -> (sum, cnt) into vals."""
                sK = slice(ta * K, tb * K)
                sT = slice(ta, tb)
                nt = tb - ta

                def seg(ap):
                    return ap.rearrange("p (t k) -> p t k", k=K)

                def bc(ap):
                    return ap[:, :, None].to_broadcast([P, nt, K])

                # cos -> d2 -> dist (rn/r precomputed per chunk)
                nc.vector.tensor_tensor(out=rn[:, sK], in0=dot_all[:, sK],
                                        in1=rn[:, sK], op=Alu.mult)
                nc.vector.tensor_tensor(out=seg(rn[:, sK]), in0=seg(rn[:, sK]),
                                        in1=bc(r_sb[:, sT]), op=Alu.mult)
                # dist = sqrt(max(2-2cos,0)+eps) == sqrt(min(cos,1)*-2+2+eps)
                nc.vector.tensor_scalar_min(rn[:, sK], rn[:, sK], 1.0)
                dist = dot_all  # dot no longer needed
                nc.scalar.activation(out=dist[:, sK], in_=rn[:, sK],
                                     func=Act.Sqrt, scale=-2.0,
                                     bias=eps_tile[:, 0:1])

                # M = -min_k dist; s = dist + M; e = exp(-s)
                nc.vector.tensor_reduce(out=M[:, sT], in_=seg(dist[:, sK]),
                                        axis=Ax.X, op=Alu.min, negate=True)
                s_t = dist
                nc.vector.tensor_tensor(out=seg(s_t[:, sK]), in0=seg(dist[:, sK]),
                                        in1=bc(M[:, sT]), op=Alu.add)
                nc.scalar.activation(out=e_t[:, sK], in_=s_t[:, sK],
                                     func=Act.Exp, scale=-1.0)

                # neg = sum(e) - sum(e*pos); under = e + neg; L = ln(under)
                nc.vector.tensor_reduce(out=sall[:, sT], in_=seg(e_t[:, sK]),
                                        axis=Ax.X, op=Alu.add)
                ep = ssn_all  # scratch
                nc.vector.tensor_tensor(out=ep[:, sK], in0=e_t[:, sK],
                                        in1=pos[:, sK], op=Alu.mult)
                nc.vector.tensor_reduce(out=spos[:, sT], in_=seg(ep[:, sK]),
                                        axis=Ax.X, op=Alu.add)
                nc.vector.tensor_tensor(out=sall[:, sT], in0=sall[:, sT],
                                        in1=spos[:, sT], op=Alu.subtract)
                nc.vector.tensor_tensor(out=seg(e_t[:, sK]), in0=seg(e_t[:, sK]),
                                        in1=bc(sall[:, sT]), op=Alu.add)
                nc.scalar.activation(out=e_t[:, sK], in_=e_t[:, sK], func=Act.Ln)

                # per_pair = L + s ; contrib = per_pair * pos * pm
                nc.vector.tensor_tensor(out=e_t[:, sK], in0=e_t[:, sK],
                                        in1=s_t[:, sK], op=Alu.add)
                nc.vector.tensor_tensor(out=seg(pos[:, sK]), in0=seg(pos[:, sK]),
                                        in1=bc(pm[:, sT]), op=Alu.mult)
                nc.vector.tensor_tensor(out=e_t[:, sK], in0=e_t[:, sK],
                                        in1=pos[:, sK], op=Alu.mult)
                nc.vector.tensor_reduce(out=vals[:, 0:1], in_=e_t[:, sK],
                                        axis=Ax.X, op=Alu.add)
                nc.vector.tensor_reduce(out=vals[:, 1:2], in_=pos[:, sK],
                                        axis=Ax.X, op=Alu.add)

            # ---- gather + per-pair dot/norm, chunk-pipelined ----
            for g in range(NCHUNK):
                grp = CHUNK_TILES[g]
                t0 = CHUNK_T0[g]
                slots = grp * K
                nidx = slots * P

                xs_t = xsp.tile([P, GGRP, C], bf16, tag="xs")
                nc.sync.dma_start(
                    out=xs_t[:, 0:grp, :],
                    in_=xs[t0 * P:(t0 + grp) * P, :]
                    .rearrange("(p t) c -> p t c", t=grp))
                idx_t = idxp.tile([P, USZ // 16], i16, tag="idx")
                nc.sync.dma_start(out=idx_t[:, 0:nidx // 16],
                                  in_=idx16[g, :, 0:nidx // 16])

                # sub-gathers; each landed slice is squared (ACT) right away
                # so the ssn tree isn't gated on a whole-chunk square pass
                nbr = nbrp.tile([P, GGRP * K, C], bf16, tag="nbr")
                sqn = sqnp.tile([P, GGRP * K, C], bf16, tag="sqn")
                for s in range(0, nidx, GMAX):
                    n = min(GMAX, nidx - s)
                    sl = slice(s // P, (s + n) // P)
                    nc.gpsimd.dma_gather(
                        out_ap=nbr[:, sl, :],
                        in_ap=tab[g, :, :],
                        idxs_ap=idx_t[:, s // 16:(s + n) // 16],
                        num_idxs=n,
                        num_idxs_reg=n,
                        elem_size=C,
                        queue_num=qcnt[0] % NQUEUES,
                    )
                    qcnt[0] += 1
                    nc.scalar.activation(out=sqn[:, sl, :], in_=nbr[:, sl, :],
                                         func=Act.Square)

                # dot(x_i, x_j): DVE bf16 product + DVE halving tree (emitted
                # first -- only gated on the gathers, not the ACT squares)
                prod = prodp.tile([P, GGRP * K, C], bf16, tag="prod")
                h = (grp // 2) * K if grp > 2 else slots
                for (a, b) in ((0, h), (h, slots)) if h < slots else ((0, slots),):
                    ta_, tb_ = a // K, b // K
                    nc.vector.tensor_tensor(
                        out=prod[:, a:b, :].rearrange("p (t k) c -> p t k c", k=K),
                        in0=xs_t[:, ta_:tb_, None, :]
                        .to_broadcast([P, tb_ - ta_, K, C]),
                        in1=nbr[:, a:b, :].rearrange("p (t k) c -> p t k c", k=K),
                        op=Alu.mult)
                halving_tree(nc.vector, prod[:, 0:slots, :],
                             dot_all[:, t0 * K:(t0 + grp) * K])

                # neighbor sum-of-squares tree (squares already landed)
                halving_tree(nc.vector, sqn[:, 0:slots, :],
                             ssn_all[:, t0 * K:(t0 + grp) * K])

                # self sum-of-squares for this chunk's points
                sqx = sqxp.tile([P, GGRP, C], bf16, tag="sqx")
                nc.scalar.activation(out=sqx[:, 0:grp, :], in_=xs_t[:, 0:grp, :],
                                     func=Act.Square)
                nc.vector.tensor_reduce(
                    out=ssi_all[:, t0:t0 + grp],
                    in_=sqx[:, 0:grp, :], axis=Ax.X, op=Alu.add)

                # inverse norms for this chunk while the stream continues
                cT = slice(t0, t0 + grp)
                cK = slice(t0 * K, (t0 + grp) * K)
                nc.vector.tensor_scalar_add(ssi_all[:, cT], ssi_all[:, cT], EPS)
                nc.vector.reciprocal(ssi_all[:, cT], ssi_all[:, cT])
                nc.scalar.activation(out=r_sb[:, cT], in_=ssi_all[:, cT],
                                     func=Act.Sqrt)
                nc.vector.tensor_scalar_add(ssn_all[:, cK], ssn_all[:, cK], EPS)
                nc.vector.reciprocal(ssn_all[:, cK], ssn_all[:, cK])
                nc.scalar.activation(out=rn[:, cK], in_=ssn_all[:, cK],
                                     func=Act.Sqrt)

                if t0 + grp == 88:
                    emit_phase_b(0, 88, vals1)

            emit_phase_b(88, T_TOT, vals2)

            # ---- final combine of the two phase-B halves ----
            nc.vector.tensor_tensor(out=vals1[:], in0=vals1[:], in1=vals2[:],
                                    op=Alu.add)
            ones = cst.tile([P, 1], f32)
            nc.vector.memset(ones[:], 1.0)
            pst = psp.tile([2, 1], f32, space="PSUM")
            nc.tensor.matmul(out=pst[:], lhsT=vals1[:], rhs=ones[:], start=True,
                             stop=True)
            res_sb = cst.tile([2, 1], f32)
            nc.vector.tensor_copy(out=res_sb[:], in_=pst[:])
            nc.sync.dma_start(out=part[:], in_=res_sb[:])
    nc.finalize()
    return nc


def make_in_maps(x, neighbor_idx, target, nsh=NSH, ncores=NCORES, fp16=False):
    """Shard + pad + per-chunk index compaction, host-side (data marshaling)."""
    import ml_dtypes

    bf = ml_dtypes.bfloat16
    x = np.ascontiguousarray(np.asarray(x, dtype=np.float32)).astype(bf)
    idx_all = np.asarray(neighbor_idx).astype(np.int64)
    tgtf = np.asarray(target).astype(np.float32)

    GGRP = max(CHUNK_TILES)
    USZ = GGRP * K * P

    in_maps = []
    for c in range(ncores):
        lo = c * nsh
        pts = np.arange(lo, lo + NPAD, dtype=np.int64)
        valid = pts < lo + nsh
        pts = np.where(valid, pts, lo)  # pad points alias point `lo`

        # point (p, t) of chunk g at xs row (t0 + t_l)*P ... laid out so the
        # chunk's block is contiguous: row = t0*P + p*grp + t_l
        xs_host = np.zeros((NPAD, C), dtype=bf)
        tgts_host = np.full((P, T_TOT), -1.0, dtype=np.float32)
        tgn_host = np.zeros((P, T_TOT, K), dtype=np.float32)
        tab_host = np.zeros((NCHUNK, USZ, C), dtype=bf)
        idx16_host = np.zeros((NCHUNK, P, USZ // 16), dtype=np.int16)

        for g, grp in enumerate(CHUNK_TILES):
            t0 = CHUNK_T0[g]
            nidx = grp * K * P
            # u in [0, grp*P): u -> (p = u//grp, t_l = u%grp)
            u = np.arange(grp * P)
            p_g, tl_g = u // grp, u % grp
            gpts = pts[t0 * P + u]
            xs_host[t0 * P + u] = x[gpts]
            t_glob = t0 + tl_g
            tgts_host[p_g, t_glob] = np.where(valid[t0 * P + u],
                                              tgtf[gpts], -1.0)
            refs = idx_all[gpts]                       # [grp*P, K]
            tgn_host[p_g, t_glob, :] = tgtf[refs]

            uniq = np.unique(refs)
            assert uniq.size <= USZ and uniq.size < 32768
            tab_host[g, :uniq.size] = x[uniq]
            lidx = np.searchsorted(uniq, refs).astype(np.int16)
            # gather position i = slot*128 + p, slot = t_l*K + k
            flat = np.zeros(nidx, dtype=np.int16)
            slot = tl_g[:, None] * K + np.arange(K)[None, :]
            flat[slot * P + p_g[:, None]] = lidx
            idx16_host[g, :, 0:nidx // 16] = np.tile(
                flat.reshape(nidx // 16, 16).T, (8, 1))

        in_maps.append({
            "xs": xs_host,
            "tab": tab_host,
            "idx16": idx16_host,
            "tgn": tgn_host.reshape(P, TK),
            "tgts": tgts_host,
        })
    return in_maps


def combine_parts(parts):
    parts = np.asarray(parts, dtype=np.float64)
    s = parts[:, 0].sum()
    cnt = parts[:, 1].sum()
    loss = s / max(cnt, 1.0) if cnt > 0 else 0.0
    return np.asarray(loss, dtype=np.float32)


def kernel(p, x, neighbor_idx, target):
    from concourse.bass_utils import run_bass_kernel_spmd

    in_maps = make_in_maps(x, neighbor_idx, target, fp16=FP16)
    nc = build_nc(fp16=FP16)
    res = run_bass_kernel_spmd(nc, in_maps, list(range(NCORES)))
    parts = [r["part"] for r in res.results]
    return combine_parts(parts)
